# revision 1
# baseline (speedup 1.0000x reference)
"""CenterLoss kernel for Trainium2 (8 NeuronCores, Bass/Tile).

Computation (reference):
    h = prelu(x, a)                      # [B, D]
    output = h @ fc3_w.T + fc3_b         # [B, C]
    c = centers[labels]                  # [B, D]
    dist = clip(sum((x - c)^2, -1), 1e-12, 1e12)
    loss = mean(dist)

Sharding: fc3_w / fc3_b are split along the class axis C across the 8
cores (tensor parallel over classes); x and the gathered center rows are
replicated.  Each core computes its [B, C/8] slice of the logits; the
(tiny) center-loss branch is computed redundantly on every core and the
result is taken from core 0.  The label gather over the full `centers`
table is an indexing-only host op; only the gathered [B, D] rows are
shipped to the device (per the sharding hint: "all-gather only the
per-label gathered center rows").

Device layout: x and the gathered centers are shipped pre-transposed
[D, B] so the contraction dim D=128 sits on SBUF partitions for both
matmul operands; fc3_w is shipped as w.T [D, C/8] for the same reason.
"""

import numpy as np

B, C, D = 2048, 100000, 128
NCORES = 8
CS = C // NCORES          # 12500 classes per core
NCHUNK = 500              # matmul moving free dim (<=512 fp32, divides CS)
CHUNKS = CS // NCHUNK     # 25
GROUP = 5                 # psum chunks per output staging tile
OUTW = NCHUNK * GROUP     # 2500 columns per output DMA
MT = B // 128             # 16 row tiles
CLAMP_MIN, CLAMP_MAX = 1e-12, 1e12

_CACHE = {}


def _build(alpha: float):
    import concourse.tile as tile
    import concourse.mybir as mybir
    from concourse import bacc

    f32 = mybir.dt.float32
    Alu = mybir.AluOpType

    nc = bacc.Bacc(
        "TRN2",
        target_bir_lowering=False,
        debug=False,
        enable_asserts=True,
        num_devices=NCORES,
    )
    xT_d = nc.dram_tensor("xT", [D, B], f32, kind="ExternalInput").ap()
    cT_d = nc.dram_tensor("cT", [D, B], f32, kind="ExternalInput").ap()
    wT_d = nc.dram_tensor("wT", [D, CS], f32, kind="ExternalInput").ap()
    bb_d = nc.dram_tensor("biasb", [128, CS], f32, kind="ExternalInput").ap()
    out_d = nc.dram_tensor("out", [B, CS], f32, kind="ExternalOutput").ap()
    loss_d = nc.dram_tensor("loss", [1, 1], f32, kind="ExternalOutput").ap()

    with tile.TileContext(nc) as tc:
        with (
            tc.tile_pool(name="persist", bufs=1) as persist,
            tc.tile_pool(name="outp", bufs=3) as outp,
            tc.tile_pool(name="mm_psum", bufs=6, space="PSUM") as mm_psum,
            tc.tile_pool(name="ls_psum", bufs=2, space="PSUM") as ls_psum,
            tc.tile_pool(name="small", bufs=1) as small,
        ):
            xT = persist.tile([D, B], f32, tag="xT")
            nc.sync.dma_start(xT[:], xT_d[:, :])
            cT = persist.tile([D, B], f32, tag="cT")
            nc.sync.dma_start(cT[:], cT_d[:, :])
            wT = persist.tile([D, CS], f32, tag="wT")
            bb = persist.tile([128, CS], f32, tag="bb")
            # Load weights/bias in column groups so the first matmuls can
            # start without waiting for the whole 6.4 MB transfer.
            for g in range(CHUNKS // GROUP):
                sl = slice(g * OUTW, (g + 1) * OUTW)
                nc.sync.dma_start(wT[:, sl], wT_d[:, sl])
                nc.sync.dma_start(bb[:, sl], bb_d[:, sl])

            # hT = prelu(xT) = max(x, 0) + alpha * min(x, 0)
            hT = persist.tile([D, B], f32, tag="hT")
            tmx = small.tile([D, B], f32, tag="tmx")
            nc.vector.tensor_scalar(hT[:], xT[:], 0.0, alpha, Alu.min, Alu.mult)
            nc.vector.tensor_scalar_max(tmx[:], xT[:], 0.0)
            nc.vector.tensor_add(hT[:], hT[:], tmx[:])

            # ---- center-loss branch ----
            sq = small.tile([D, B], f32, tag="sq")
            nc.vector.tensor_tensor(sq[:], xT[:], cT[:], Alu.subtract)
            nc.vector.tensor_mul(sq[:], sq[:], sq[:])
            ones = small.tile([128, 1], f32, tag="ones")
            nc.vector.memset(ones[:], 1.0)
            dist = small.tile([1, B], f32, tag="dist")
            for q in range(B // 512):
                ps = ls_psum.tile([1, 512], f32, tag="lps")
                nc.tensor.matmul(
                    ps[:],
                    ones[:, 0:1],
                    sq[:, q * 512:(q + 1) * 512],
                    start=True,
                    stop=True,
                )
                nc.vector.tensor_copy(out=dist[0:1, q * 512:(q + 1) * 512], in_=ps[:])
            nc.vector.tensor_scalar(
                dist[:], dist[:], CLAMP_MIN, CLAMP_MAX, Alu.max, Alu.min
            )
            lsum = small.tile([1, 1], f32, tag="lsum")
            nc.vector.reduce_sum(lsum[0:1, 0:1], dist[0:1, :], axis=mybir.AxisListType.X)
            nc.vector.tensor_scalar_mul(lsum[:], lsum[:], 1.0 / B)
            nc.sync.dma_start(loss_d[:, :], lsum[:])

            # ---- classifier branch: out = h @ w.T + bias ----
            for t in range(MT):
                lhsT = hT[:, t * 128:(t + 1) * 128]
                for g in range(CHUNKS // GROUP):
                    ot = outp.tile([128, OUTW], f32, tag="ot")
                    for j in range(GROUP):
                        n0 = (g * GROUP + j) * NCHUNK
                        ps = mm_psum.tile([128, NCHUNK], f32, tag="mmps")
                        nc.tensor.matmul(
                            ps[:],
                            lhsT,
                            wT[:, n0:n0 + NCHUNK],
                            start=True,
                            stop=True,
                        )
                        nc.vector.tensor_tensor(
                            ot[:, j * NCHUNK:(j + 1) * NCHUNK],
                            ps[:],
                            bb[:, n0:n0 + NCHUNK],
                            Alu.add,
                        )
                    nc.sync.dma_start(
                        out_d[t * 128:(t + 1) * 128, g * OUTW:(g + 1) * OUTW],
                        ot[:],
                    )

    nc.compile()
    return nc


def _run(inputs, trace=False, trace_cores=None):
    from concourse.bass_utils import run_bass_kernel_spmd

    x = np.ascontiguousarray(np.asarray(inputs["x"], dtype=np.float32))
    centers = np.asarray(inputs["centers"], dtype=np.float32)
    prelu_a = np.asarray(inputs["prelu_a"], dtype=np.float32)
    fc3_w = np.asarray(inputs["fc3_w"], dtype=np.float32)
    fc3_b = np.asarray(inputs["fc3_b"], dtype=np.float32)
    labels = np.asarray(inputs["labels"])

    alpha = float(prelu_a.reshape(-1)[0])

    xT = np.ascontiguousarray(x.T)                       # [D, B]
    cT = np.ascontiguousarray(centers[labels].T)         # [D, B]

    in_maps = []
    for m in range(NCORES):
        wm = fc3_w[m * CS:(m + 1) * CS, :]               # [CS, D]
        bm = fc3_b[m * CS:(m + 1) * CS]                  # [CS]
        in_maps.append({
            "xT": xT,
            "cT": cT,
            "wT": np.ascontiguousarray(wm.T),            # [D, CS]
            "biasb": np.ascontiguousarray(np.broadcast_to(bm, (128, CS))),
        })

    key = alpha
    if key not in _CACHE:
        _CACHE[key] = _build(alpha)
    nc = _CACHE[key]

    res = run_bass_kernel_spmd(
        nc,
        in_maps,
        core_ids=list(range(NCORES)),
        trace=trace,
        trace_cores=trace_cores,
    )
    output = np.concatenate([r["out"] for r in res.results], axis=1)
    loss = np.asarray(res.results[0]["loss"], dtype=np.float32).reshape(())
    return (loss, output), res


def kernel(**inputs):
    (loss, output), _ = _run(inputs, trace=False)
    return loss, output
